# revision 12
# baseline (speedup 1.0000x reference)
"""Trainium2 Bass kernel for nn_FlexibleGATNet (GATv2 x2 + MLP head + mean-pool).

Sharding: nodes split into 8 contiguous ranges of 2500 (one per NeuronCore);
every edge is owned by the core that owns its dst node, so the scatter-softmax
over dst needs no cross-core reduce. Edges are sorted by dst and grouped into
128-node chunks; per chunk, gathered source features are combined via
PSUM-accumulated one-hot matmuls. xl tables for layer 1 are exchanged with an
AllGather. The small weights are replicated.
"""

import sys
import numpy as np

sys.path.insert(0, "/opt/trn_rl_repo")

# ---- problem dims (hardcoded per spec) ----
N, E, G = 20000, 320000, 64
IN, ED, H, F, NC = 6, 2, 4, 64, 2
HF = H * F  # 256
FC = 64
NCORE = 8
NLOC = N // NCORE          # 2500 nodes per core
CH = 20                    # node chunks per core (20*128 = 2560 >= 2500)
NPAD = CH * 128            # padded local node count (2560)
GROW = NPAD                # row stride of a core slice inside allgathered tables
P = 128

_CACHE = {}


# ------------------------------------------------------------------ host prep
def _wrap16(idx):
    """dma_gather index layout: [128, n/16] int16, idx j at [j%16, j//16],
    replicated across the 8 groups of 16 partitions."""
    a = idx.astype(np.int16).reshape(-1, 16).T  # [16, cols]
    return np.ascontiguousarray(np.tile(a, (8, 1)))


def _slot_cols(vals, fill):
    """per-slot array -> [128, nslots/128] (slot s at [s%128, s//128])."""
    a = vals.reshape(-1, 128).T
    return np.ascontiguousarray(a.astype(np.float32))


def _preprocess(inputs):
    x = np.asarray(inputs["x"], np.float32)
    ei = np.asarray(inputs["edge_index"], np.int64)
    ea = np.asarray(inputs["edge_attr"], np.float32)
    batch = np.asarray(inputs["batch"], np.int64)
    src0, dst0 = ei[0], ei[1]

    # self loops with mean incoming edge_attr (PyG fill_value='mean')
    s = np.zeros((N, ED), np.float32)
    np.add.at(s, dst0, ea)
    cnt = np.bincount(dst0, minlength=N).astype(np.float32)
    loop_attr = s / np.maximum(cnt, 1.0)[:, None]
    idx = np.arange(N, dtype=np.int64)
    src2 = np.concatenate([src0, idx])
    dst2 = np.concatenate([dst0, idx])
    ea2 = np.concatenate([ea, loop_attr], 0)

    core_of = dst2 // NLOC
    # per (core, chunk) sub counts, then shared max per chunk (program is SPMD)
    percore = []
    for c in range(NCORE):
        sel = np.nonzero(core_of == c)[0]
        dloc = (dst2[sel] - c * NLOC).astype(np.int64)
        order = np.argsort(dloc, kind="stable")
        sel = sel[order]
        dloc = dloc[order]
        bounds = np.searchsorted(dloc, np.arange(0, NPAD + 1, 128))
        percore.append((sel, dloc, bounds))
    nsub = []
    for k in range(CH):
        m = max(pc[2][k + 1] - pc[2][k] for pc in percore)
        nsub.append(max(1, int(-(-m // 128))))
    totsub = sum(nsub)
    totslot = totsub * 128

    per_core_arrays = []
    for c in range(NCORE):
        sel, dloc, bounds = percore[c]
        srcrow = np.zeros(totslot, np.int64)
        dstloc = np.zeros(totslot, np.int64)
        col = np.full(totslot, -1.0, np.float32)
        eaT = np.zeros((3, totslot), np.float32)
        pos = 0
        for k in range(CH):
            lo, hi = bounds[k], bounds[k + 1]
            n_k = hi - lo
            sl = sel[lo:hi]
            sr = src2[sl]
            srcrow[pos:pos + n_k] = (sr // NLOC) * GROW + (sr % NLOC)
            dstloc[pos:pos + n_k] = dloc[lo:hi]
            col[pos:pos + n_k] = (dloc[lo:hi] - k * 128).astype(np.float32)
            eaT[0, pos:pos + n_k] = ea2[sl, 0]
            eaT[1, pos:pos + n_k] = ea2[sl, 1]
            eaT[2, pos:pos + n_k] = 1.0
            pos += nsub[k] * 128
        # graph column per local node (chunk-major [128, CH]), -1 for pads
        gcol = np.full((P, CH), -1.0, np.float32)
        for k in range(CH):
            glob = c * NLOC + k * 128 + np.arange(128)
            ok = glob < (c + 1) * NLOC
            gcol[ok, k] = batch[glob[ok]].astype(np.float32)
        # local node features, transposed + ones row
        xT = np.zeros((IN + 1, NPAD), np.float32)
        xT[:IN, :NLOC] = x[c * NLOC:(c + 1) * NLOC].T
        xT[IN, :NLOC] = 1.0
        per_core_arrays.append(dict(
            src16=_wrap16(srcrow),
            dst16=_wrap16(dstloc),
            dstcol=_slot_cols(col, -1.0),
            eaT=eaT,
            gcol=gcol,
            xT=xT,
        ))
    return per_core_arrays, nsub, batch


def _weights(inputs):
    w = {}
    z1 = np.zeros((1, HF), np.float32)
    w["W0l"] = np.concatenate([np.asarray(inputs["Wl0"], np.float32), z1], 0)
    w["W0r"] = np.concatenate([np.asarray(inputs["Wr0"], np.float32), z1], 0)
    w["P0"] = np.concatenate([np.asarray(inputs["proj0"], np.float32),
                              -np.ones((1, HF), np.float32)], 0)
    bl0 = np.asarray(inputs["bl0"], np.float32) + np.asarray(inputs["br0"], np.float32)
    bl1 = np.asarray(inputs["bl1"], np.float32) + np.asarray(inputs["br1"], np.float32)
    w["eeW0"] = np.concatenate([np.asarray(inputs["We0"], np.float32), bl0[None]], 0)
    w["eeW1"] = np.concatenate([np.asarray(inputs["We1"], np.float32), bl1[None]], 0)
    w["att0"] = np.tile(np.asarray(inputs["att0"], np.float32).reshape(1, HF), (P, 1))
    w["att1"] = np.tile(np.asarray(inputs["att1"], np.float32).reshape(1, HF), (P, 1))
    w["b0"] = np.tile(np.asarray(inputs["b0"], np.float32).reshape(1, HF), (P, 1))
    w["b1"] = np.tile(np.asarray(inputs["b1"], np.float32).reshape(1, HF), (P, 1))
    def kchunk(a):
        a = np.asarray(a, np.float32)
        return np.ascontiguousarray(a.reshape(2, 128, a.shape[1]).transpose(1, 0, 2))
    w["Wl1"] = kchunk(inputs["Wl1"])
    w["Wr1"] = kchunk(inputs["Wr1"])
    w["Wp"] = kchunk(inputs["Wp"])
    w["bp"] = np.asarray(inputs["bp"], np.float32).reshape(1, FC)
    w["iota"] = np.tile(np.arange(P, dtype=np.float32), (P, 1))
    w["ident"] = np.eye(P, dtype=np.float32)
    return w


# ------------------------------------------------------------------ device
def _build(nsub, stage=4):
    import concourse.tile as tile
    from concourse import bacc, mybir

    dt = mybir.dt
    totsub = sum(nsub)
    totslot = totsub * 128
    MAXSUB = max(nsub)

    nc = bacc.Bacc("TRN2", target_bir_lowering=False, debug=False,
                   num_devices=NCORE)

    # ---- external IO ----
    T = {}
    def inp(name, shape, d=dt.float32):
        T[name] = nc.dram_tensor(name, shape, d, kind="ExternalInput")
        return T[name]

    inp("src16", [P, totslot // 16], dt.int16)
    inp("dst16", [P, totslot // 16], dt.int16)
    inp("dstcol", [P, totsub])
    inp("eaT", [3, totslot])
    inp("gcol", [P, CH])
    inp("xT", [IN + 1, NPAD])
    for nm, sh in [("W0l", [IN + 1, HF]), ("W0r", [IN + 1, HF]),
                   ("P0", [IN + 1, HF]), ("eeW0", [3, HF]), ("eeW1", [3, HF]),
                   ("att0", [P, HF]), ("att1", [P, HF]), ("b0", [P, HF]),
                   ("b1", [P, HF]), ("Wl1", [P, 2, HF]), ("Wr1", [P, 2, HF]),
                   ("Wp", [P, 2, FC]), ("bp", [1, FC]), ("iota", [P, P]),
                   ("ident", [P, P])]:
        inp(nm, sh)
    O_pool = nc.dram_tensor("pooled", [G, FC], dt.float32, kind="ExternalOutput")
    O_dbg = None
    if stage != 4:
        O_dbg = nc.dram_tensor("dbg", [P, CH * HF], dt.float32,
                               kind="ExternalOutput")

    # ---- internal DRAM tables ----
    xl0loc = nc.dram_tensor("xl0loc", [NPAD, HF], dt.float32)
    xr0loc = nc.dram_tensor("xr0loc", [NPAD, HF], dt.float32)
    xl1loc = nc.dram_tensor("xl1loc", [NPAD, HF], dt.float32)
    xr1loc = nc.dram_tensor("xr1loc", [NPAD, HF], dt.float32)
    xl0full = nc.dram_tensor("xl0full", [NCORE * NPAD, HF], dt.float32,
                             addr_space="Shared")
    xl1full = nc.dram_tensor("xl1full", [NCORE * NPAD, HF], dt.float32,
                             addr_space="Shared")

    from concourse.bass import AP as _AP

    def _midb(ap, w):
        return _AP(ap.tensor, ap.offset,
                   [list(ap.ap[0]), [0, w], list(ap.ap[1])])

    AF = mybir.ActivationFunctionType
    AL = mybir.AluOpType
    RG = [list(range(NCORE))]

    with tile.TileContext(nc) as tc:
        with tc.tile_pool(name="const", bufs=1) as cpool, \
             tc.tile_pool(name="idxp", bufs=1) as ipool, \
             tc.tile_pool(name="gat", bufs=2) as gpool, \
             tc.tile_pool(name="work", bufs=2) as wpool, \
             tc.tile_pool(name="node", bufs=1) as npool, \
             tc.tile_pool(name="ps", bufs=2, space="PSUM") as psp, \
             tc.tile_pool(name="ps1", bufs=1, space="PSUM") as psp1, \
             tc.tile_pool(name="psacc", bufs=1, space="PSUM") as psa:

            # ---- load constants ----
            C = {}
            for nm, sh in [("eeW0", [3, HF]), ("eeW1", [3, HF]),
                           ("att0", [P, HF]), ("att1", [P, HF]),
                           ("b0", [P, HF]), ("b1", [P, HF]),
                           ("W0l", [IN + 1, HF]), ("W0r", [IN + 1, HF]),
                           ("P0", [IN + 1, HF]), ("Wl1", [P, 2, HF]),
                           ("Wr1", [P, 2, HF]), ("Wp", [P, 2, FC]), ("bp", [1, FC]),
                           ("iota", [P, P]), ("ident", [P, P]),
                           ("xT", [IN + 1, NPAD]), ("gcol", [P, CH])]:
                C[nm] = cpool.tile(sh, dt.float32, tag=nm, name="c_" + nm)
                nc.sync.dma_start(out=C[nm][:], in_=T[nm].ap())
            ones1 = cpool.tile([1, P], dt.float32, tag="ones1")
            nc.vector.memset(ones1[:], 1.0)

            src16 = ipool.tile([P, totslot // 16], dt.int16, tag="src16")
            dst16 = ipool.tile([P, totslot // 16], dt.int16, tag="dst16")
            dstcol = ipool.tile([P, totsub], dt.float32, tag="dstcol")
            nc.sync.dma_start(out=src16[:], in_=T["src16"].ap())
            nc.sync.dma_start(out=dst16[:], in_=T["dst16"].ap())
            nc.sync.dma_start(out=dstcol[:], in_=T["dstcol"].ap())

            x1_all = npool.tile([P, CH, HF], dt.float32, tag="x1all")

            # ---- phase T0: build L0 tables (local slices) ----
            for k in range(CH):
                xk = C["xT"][:, k * 128:(k + 1) * 128]
                for tab, Wn in ((xl0loc, "W0l"), (xr0loc, "W0r")):
                    ps = psp1.tile([P, HF], dt.float32, space="PSUM", tag="scr")
                    nc.tensor.matmul(ps[:], lhsT=xk, rhs=C[Wn][:],
                                     start=True, stop=True)
                    sb = wpool.tile([P, HF], dt.float32, tag="tsb")
                    nc.scalar.activation(out=sb[:], in_=ps[:], func=AF.Copy)
                    nc.sync.dma_start(out=tab.ap()[k * 128:(k + 1) * 128],
                                      in_=sb[:])
            nc.gpsimd.collective_compute(
                "AllGather", AL.bypass, replica_groups=RG,
                ins=[xl0loc.ap()], outs=[xl0full.ap()])

            # ---- edge + node phase (shared for both layers) ----
            def layer(lyr, xlfull, xrloc, eeW, attW, bW, sstage=99):
                sub0 = 0
                for k in range(CH):
                    ns = nsub[k]
                    nslot = ns * 128
                    c0 = sub0 * 8  # int16 col offset (128 slots -> 8 cols)
                    eak = gpool.tile([3, MAXSUB * 128], dt.float32, tag="eak")
                    nc.sync.dma_start(
                        out=eak[:, :nslot],
                        in_=T["eaT"].ap()[:, sub0 * 128:sub0 * 128 + nslot])
                    xl_g = gpool.tile([P, MAXSUB, HF], dt.float32, tag="xlg")
                    xr_g = gpool.tile([P, MAXSUB, HF], dt.float32, tag="xrg")
                    for s0 in range(0, ns, 8):
                        s1 = min(s0 + 8, ns)
                        nw = (s1 - s0) * 128
                        nc.gpsimd.dma_gather(
                            out_ap=xl_g[:, s0:s1, :], in_ap=xlfull.ap(),
                            idxs_ap=src16[:, c0 + s0 * 8:c0 + s1 * 8],
                            num_idxs=nw, num_idxs_reg=nw, elem_size=HF)
                        nc.gpsimd.dma_gather(
                            out_ap=xr_g[:, s0:s1, :], in_ap=xrloc.ap(),
                            idxs_ap=dst16[:, c0 + s0 * 8:c0 + s1 * 8],
                            num_idxs=nw, num_idxs_reg=nw, elem_size=HF)

                    acc = psa.tile([P, HF], dt.float32, space="PSUM",
                                   tag="acc")
                    dacc = psa.tile([P, H], dt.float32, space="PSUM",
                                    tag="dacc")
                    out_ps = acc[:]
                    den_ps = dacc[:]
                    if sstage <= 15:
                        sub0 += ns
                        continue
                    ntile = -(-ns // 4)
                    for t in range(ntile):
                        s_lo = t * 4
                        s_hi = min(s_lo + 4, ns)
                        w = s_hi - s_lo
                        pp = psp.tile([P, 4, HF], dt.float32, space="PSUM",
                                      tag="pp")
                        for s in range(s_lo, s_hi):
                            g0 = s * 128
                            pps = pp[:, s - s_lo, :]
                            nc.tensor.matmul(
                                pps, lhsT=eak[:, g0:g0 + 128], rhs=C[eeW][:],
                                start=True, stop=False)
                            nc.tensor.matmul(
                                pps, lhsT=C["ident"][:], rhs=xl_g[:, s, :],
                                start=False, stop=False)
                            nc.tensor.matmul(
                                pps, lhsT=C["ident"][:], rhs=xr_g[:, s, :],
                                start=False, stop=True)
                        mlr = wpool.tile([P, 4, HF], dt.float32, tag="mlr")
                        nc.scalar.activation(out=mlr[:, :w, :], in_=pp[:, :w, :],
                                             func=AF.Prelu, alpha=0.2)
                        if sstage <= 16:
                            continue
                        am = wpool.tile([P, 4, HF], dt.float32, tag="am")
                        nc.vector.tensor_tensor(
                            out=am[:, :w, :], in0=mlr[:, :w, :],
                            in1=_midb(C[attW][:], w),
                            op=AL.mult)
                        lg = wpool.tile([P, 4, H], dt.float32, tag="lg")
                        nc.vector.tensor_reduce(
                            out=lg[:, :w, :],
                            in_=am[:, :w, :].rearrange("p s (h f) -> p s h f", h=H),
                            axis=mybir.AxisListType.X, op=AL.add)
                        ex = wpool.tile([P, 4, H], dt.float32, tag="ex")
                        nc.scalar.activation(out=ex[:, :w, :], in_=lg[:, :w, :],
                                             func=AF.Exp)
                        exe = wpool.tile([P, 4, HF], dt.float32, tag="exe")
                        nc.vector.tensor_copy(
                            out=exe[:, :w, :].rearrange(
                                "p s (h f) -> p s h f", h=H),
                            in_=ex[:, :w, :].broadcast_to([P, w, H, F]))
                        v = wpool.tile([P, 4, HF], dt.float32, tag="v")
                        nc.vector.tensor_tensor(out=v[:, :w, :],
                                                in0=xl_g[:, s_lo:s_hi, :],
                                                in1=exe[:, :w, :], op=AL.mult)
                        if sstage <= 17:
                            continue
                        for s in range(s_lo, s_hi):
                            oh = wpool.tile([P, P], dt.float32, tag="oh")
                            nc.vector.tensor_scalar(
                                out=oh[:], in0=C["iota"][:],
                                scalar1=dstcol[:, sub0 + s:sub0 + s + 1],
                                scalar2=None, op0=AL.is_equal)
                            first = s == 0
                            last = s == ns - 1
                            nc.tensor.matmul(out_ps, lhsT=oh[:],
                                             rhs=v[:, s - s_lo, :],
                                             start=first, stop=last)
                            nc.tensor.matmul(den_ps, lhsT=oh[:],
                                             rhs=ex[:, s - s_lo, :],
                                             start=first, stop=last)

                    # ---- node phase for chunk k ----
                    if sstage <= 18:
                        sub0 += ns
                        continue
                    den = wpool.tile([P, H], dt.float32, tag="den")
                    nc.vector.tensor_scalar(out=den[:], in0=den_ps,
                                            scalar1=1e-30, scalar2=None,
                                            op0=AL.max)
                    rec = wpool.tile([P, H], dt.float32, tag="rec")
                    nc.vector.reciprocal(out=rec[:], in_=den[:])
                    ht = wpool.tile([P, HF], dt.float32, tag="ht")
                    for h in range(H):
                        nc.vector.tensor_tensor(
                            out=ht[:, h * F:(h + 1) * F],
                            in0=out_ps[:, h * F:(h + 1) * F],
                            in1=rec[:, h:h + 1].to_broadcast([P, F]),
                            op=AL.mult)
                    hb = wpool.tile([P, HF], dt.float32, tag="hb")
                    nc.vector.tensor_tensor(out=hb[:], in0=ht[:], in1=C[bW][:],
                                            op=AL.add)
                    # elu(hb) = relu(hb) + exp(min(hb,0)) - 1
                    r = wpool.tile([P, HF], dt.float32, tag="relu")
                    nc.scalar.activation(out=r[:], in_=hb[:], func=AF.Relu)
                    neg = wpool.tile([P, HF], dt.float32, tag="neg")
                    nc.vector.tensor_tensor(out=neg[:], in0=hb[:], in1=r[:],
                                            op=AL.subtract)
                    e = wpool.tile([P, HF], dt.float32, tag="eexp")
                    nc.scalar.activation(out=e[:], in_=neg[:], func=AF.Exp)

                    if lyr == 0:
                        # x1 = elu + x@proj0 ; P0 ones-row = -1 bakes in the -1
                        xp_ps = psp1.tile([P, HF], dt.float32, space="PSUM",
                                          tag="scr")
                        nc.tensor.matmul(
                            xp_ps[:], lhsT=C["xT"][:, k * 128:(k + 1) * 128],
                            rhs=C["P0"][:], start=True, stop=True)
                        t1 = wpool.tile([P, HF], dt.float32, tag="t1")
                        nc.vector.tensor_tensor(out=t1[:], in0=r[:], in1=e[:],
                                                op=AL.add)
                        nc.vector.tensor_tensor(out=x1_all[:, k, :], in0=t1[:],
                                                in1=xp_ps[:], op=AL.add)
                    else:
                        # x2 = elu + x1 ; then head: z = elu(x2@Wp+bp), pool
                        t1 = wpool.tile([P, HF], dt.float32, tag="t1")
                        nc.vector.tensor_tensor(out=t1[:], in0=r[:], in1=e[:],
                                                op=AL.add)
                        x2 = wpool.tile([P, HF], dt.float32, tag="x2")
                        nc.vector.tensor_scalar(out=x2[:], in0=t1[:],
                                                scalar1=-1.0, scalar2=None,
                                                op0=AL.add)
                        nc.vector.tensor_tensor(out=x2[:], in0=x2[:],
                                                in1=x1_all[:, k, :], op=AL.add)
                        # transpose x2 (2 f-chunks) then z = x2 @ Wp + bp
                        x2T = wpool.tile([P, 2, P], dt.float32, tag="x2T")
                        for j in range(2):
                            tp = psp1.tile([P, P], dt.float32, space="PSUM",
                                           tag="scr")
                            nc.tensor.transpose(
                                out=tp[:], in_=x2[:, j * 128:(j + 1) * 128],
                                identity=C["ident"][:])
                            nc.vector.tensor_copy(out=x2T[:, j, :], in_=tp[:])
                        z_ps = psp1.tile([P, FC], dt.float32, space="PSUM",
                                         tag="scr")
                        for j in range(2):
                            nc.tensor.matmul(z_ps[:], lhsT=x2T[:, j, :],
                                             rhs=C["Wp"][:, j, :],
                                             start=(j == 0), stop=False)
                        nc.tensor.matmul(z_ps[:], lhsT=ones1[:], rhs=C["bp"][:],
                                         start=False, stop=True)
                        zr = wpool.tile([P, FC], dt.float32, tag="zr")
                        nc.scalar.activation(out=zr[:], in_=z_ps[:], func=AF.Relu)
                        zneg = wpool.tile([P, FC], dt.float32, tag="zneg")
                        nc.vector.tensor_tensor(out=zneg[:], in0=z_ps[:],
                                                in1=zr[:], op=AL.subtract)
                        ze = wpool.tile([P, FC], dt.float32, tag="ze")
                        nc.scalar.activation(out=ze[:], in_=zneg[:], func=AF.Exp)
                        z = wpool.tile([P, FC], dt.float32, tag="z")
                        nc.vector.tensor_tensor(out=z[:], in0=zr[:], in1=ze[:],
                                                op=AL.add)
                        nc.vector.tensor_scalar(out=z[:], in0=z[:], scalar1=-1.0,
                                                scalar2=None, op0=AL.add)
                        ohg = wpool.tile([P, G], dt.float32, tag="ohg")
                        nc.vector.tensor_scalar(
                            out=ohg[:], in0=C["iota"][:, :G],
                            scalar1=C["gcol"][:, k:k + 1], scalar2=None,
                            op0=AL.is_equal)
                        nc.tensor.matmul(pool_ps[:], lhsT=ohg[:], rhs=z[:],
                                         start=(k == 0), stop=(k == CH - 1))
                    sub0 += ns

            pool_ps = psa.tile([G, FC], dt.float32, space="PSUM",
                               tag="poolacc")

            if stage >= 2:
                layer(0, xl0full, xr0loc, "eeW0", "att0", "b0",
                      sstage=(stage if 15 <= stage <= 18 else 99))

            if stage < 4:
                if stage >= 2 and not (15 <= stage <= 18):
                    nc.sync.dma_start(out=O_dbg.ap(),
                                      in_=x1_all[:].rearrange("p c f -> p (c f)"))
                else:
                    nc.sync.dma_start(
                        out=O_dbg.ap().rearrange("p (c f) -> (p c) f", f=HF),
                        in_=xl0full.ap()[0:P * CH, :])

            if stage in (3, 4):
                # ---- phase T1: build L1 tables from x1 ----
                for k in range(CH):
                    x1T = wpool.tile([P, 2, P], dt.float32, tag="x1T")
                    for j in range(2):
                        tp = psp1.tile([P, P], dt.float32, space="PSUM", tag="scr")
                        nc.tensor.transpose(
                            out=tp[:], in_=x1_all[:, k, j * 128:(j + 1) * 128],
                            identity=C["ident"][:])
                        nc.vector.tensor_copy(out=x1T[:, j, :], in_=tp[:])
                    for tab, Wn in ((xl1loc, "Wl1"), (xr1loc, "Wr1")):
                        ps = psp1.tile([P, HF], dt.float32, space="PSUM", tag="scr")
                        for j in range(2):
                            nc.tensor.matmul(ps[:], lhsT=x1T[:, j, :],
                                             rhs=C[Wn][:, j, :],
                                             start=(j == 0), stop=(j == 1))
                        sb = wpool.tile([P, HF], dt.float32, tag="tsb")
                        nc.scalar.activation(out=sb[:], in_=ps[:], func=AF.Copy)
                        nc.sync.dma_start(out=tab.ap()[k * 128:(k + 1) * 128],
                                          in_=sb[:])
                nc.gpsimd.collective_compute(
                    "AllGather", AL.bypass, replica_groups=RG,
                    ins=[xl1loc.ap()], outs=[xl1full.ap()])

            if stage == 4:
                layer(1, xl1full, xr1loc, "eeW1", "att1", "b1")

            pool_sb = wpool.tile([G, FC], dt.float32, tag="poolsb")
            if stage == 4:
                nc.vector.tensor_copy(out=pool_sb[:], in_=pool_ps[:])
            else:
                nc.vector.memset(pool_sb[:], 0.0)
            nc.sync.dma_start(out=O_pool.ap(), in_=pool_sb[:])

    nc.compile()
    return nc


# ------------------------------------------------------------------ entry
def kernel(**inputs):
    from concourse.bass_utils import run_bass_kernel_spmd

    per_core, nsub, batch = _preprocess(inputs)
    w = _weights(inputs)

    import os
    stage = int(os.environ.get("GAT_STAGE", "4"))
    key = (tuple(nsub), stage)
    if key not in _CACHE:
        _CACHE[key] = _build(nsub, stage)
    nc = _CACHE[key]

    in_maps = []
    for c in range(NCORE):
        m = dict(per_core[c])
        m.update(w)
        in_maps.append(m)
    res = run_bass_kernel_spmd(nc, in_maps, core_ids=list(range(NCORE)))

    pooled_sum = np.zeros((G, FC), np.float32)
    for c in range(NCORE):
        pooled_sum += res.results[c]["pooled"]
    gc = np.bincount(np.asarray(inputs["batch"], np.int64), minlength=G)
    pooled = pooled_sum / np.maximum(gc, 1.0)[:, None].astype(np.float32)
    out = pooled @ np.asarray(inputs["Wc"], np.float32) + np.asarray(
        inputs["bc"], np.float32)
    return out.astype(np.float32)
